# revision 14
# baseline (speedup 1.0000x reference)
"""Trainium2 Bass kernel for nn_ContrastiveDist (supervised contrastive loss).

Math
----
The (n,n) distance/weight matrices collapse to per-class statistics.  With
classes c = 0..15, per-class count cnt[c], feature sums C[c,:], squared-norm
sums SqS[c], global sums Ftot / SSall:

    alpha[c] = 1/(cnt[c]-1+eps),  beta[c] = 1/(n-cnt[c]+eps)
    loss_i   = f_i . R[c_i] + sq_i*P[c_i] + (Q[c_i]+M)
      R[c,:] = 2*beta*(Ftot-C[c]) - 2*alpha*C[c]
      P[c]   = alpha*cnt - beta*(n-cnt)
      Q[c]   = alpha*SqS[c] - beta*(SSall-SqS[c])
    result   = sum(relu(loss_i)*valid_i) / max(sum(valid_i), 1)

valid_i = (cnt[c_i] >= 2) is folded into the coefficients (R/P/QM rows of
invalid classes zeroed -> relu(loss)=0 there).

Device pipeline (fp8e4 features, ~1e-4 rel err vs f32 reference; errors are
row/element-wise symmetric roundings that average out over 8192 rows):
  1. two interleaved 64-matmul PSUM chains over the row tiles produce
     statsT(128d,16c) = sum_t fh_t^T @ onehot_t  (fp8 x fp8) and
     sqstatsT(128d,16c) = sum_t (fh_t^2)^T @ onehot_t  (bf16 x bf16),
     overlapped with the feature DMA.
  2. cnt-only coefficients (alpha/beta/vmask/P and their 128-partition
     broadcast via a ones(1,128) rank-1 matmul) are computed EARLY from the
     one-hot column sums; only QM (SqS) and RT (stats) trail the DMA.
  3. loss:   per 512-col chunk, PSUM = RT^T @ fT + P128^T @ fT^2 (all fp8;
     the second matmul realizes P[c]*sq_i since sum_d fT^2[d,i] = sq_i),
     then relu(PSUM + QM[c]) on the scalar engine and mask*accumulate on
     vector.
HBM traffic ~2.8MB/core; every core computes redundantly (no collectives).
Notes: DVE (MULTIPLY, BYPASS) tensor_scalar is pathologically slow; DVE
stride-0 broadcast operands are ~100x slow (use Act per-partition bias
instead); dma_start stalls the issuing sequencer while its HWDGE ring is
busy, so the Act ring gets exactly one transfer and Act's compute starts
only after that single trigger.
"""

import numpy as np
import ml_dtypes

import concourse.bacc as bacc
import concourse.tile as tile
import concourse.mybir as mybir
from concourse.bass_utils import run_bass_kernel_spmd

N, D, K, NCORES = 8192, 128, 16, 8
T = N // 128               # 64 row-tiles of 128
NCH = 16                   # dot chunks of 512 cols
CH = N // NCH
FCH = 4                    # square chunking (2048 cols each)
EPS, MARGIN = 1e-6, 10.0
F32 = mybir.dt.float32
BF16 = mybir.dt.bfloat16
FP8 = mybir.dt.float8e4
Alu = mybir.AluOpType
Act = mybir.ActivationFunctionType
AxX = mybir.AxisListType.X

_CACHE: dict = {}


def _build():
    if "nc" in _CACHE:
        return _CACHE["nc"]

    nc = bacc.Bacc("TRN2", target_bir_lowering=False, debug=False, num_devices=NCORES)
    fhr = nc.dram_tensor("fhr", [128, T * D], FP8, kind="ExternalInput").ap()
    ftr = nc.dram_tensor("ftr", [128, N], FP8, kind="ExternalInput").ap()
    eohr8 = nc.dram_tensor("eohr8", [128, T * K], FP8, kind="ExternalInput").ap()
    eohrb = nc.dram_tensor("eohrb", [128, T * K], BF16, kind="ExternalInput").ap()
    eoht = nc.dram_tensor("eoht", [K, N], BF16, kind="ExternalInput").ap()
    res = nc.dram_tensor("res", [1, 1], F32, kind="ExternalOutput").ap()

    with tile.TileContext(nc) as tc:
        with (
            tc.tile_pool(name="sb", bufs=1) as sb,
            tc.tile_pool(name="ps", bufs=1, space="PSUM") as ps,
        ):
            # ------------- loads: sync + gpsimd rings, one DMA on Act ring ------
            eohb = sb.tile([128, T * K], BF16)
            eoh8 = sb.tile([128, T * K], FP8)
            fh = sb.tile([128, T * D], FP8)
            ft = sb.tile([128, N], FP8)
            eohts = sb.tile([K, N], BF16)
            HF = T * D // 2
            HT = N // 2
            # Act ring: exactly one transfer (first ft half, needed earliest
            # of the late tensors); a single trigger does not stall Act.
            nc.scalar.dma_start(ft[:, 0:HT], ftr[:, 0:HT])
            nc.sync.dma_start(fh[:, 0:HF], fhr[:, 0:HF])
            nc.gpsimd.dma_start(fh[:, HF:2 * HF], fhr[:, HF:2 * HF])
            nc.sync.dma_start(eohb[:], eohrb)
            nc.gpsimd.dma_start(eoh8[:], eohr8)
            nc.gpsimd.dma_start(eohts[:], eoht)
            nc.sync.dma_start(ft[:, HT:2 * HT], ftr[:, HT:2 * HT])

            fh3 = fh.rearrange("p (t d) -> p t d", d=D)
            eoh83 = eoh8.rearrange("p (t c) -> p t c", c=K)
            eohb3 = eohb.rearrange("p (t c) -> p t c", c=K)
            eohb3c = eohb.rearrange("p (t c) -> p c t", c=K)

            ones128 = sb.tile([128, 1], F32)
            nc.gpsimd.memset(ones128[:], 1.0)
            ones1 = sb.tile([1, 128], F32)
            nc.gpsimd.memset(ones1[:], 1.0)
            # preload the Relu activation table off the critical path
            dumm = sb.tile([1, 1], BF16)
            nc.scalar.activation(dumm[:], ones1[:, 0:1], Act.Relu)

            # ------------- early: cnt and cnt-only coefficients ----------------
            cntpart = sb.tile([128, K], F32)
            nc.vector.tensor_reduce(cntpart[:], eohb3c, axis=AxX, op=Alu.add)
            cntP = ps.tile([1, K], F32, tag="cntP", bufs=1, name="cntP")
            nc.tensor.matmul(cntP[:], ones128[:], cntpart[:], start=True, stop=True,
                             skip_group_check=True)
            cntf = sb.tile([1, K], F32)
            nc.vector.tensor_copy(cntf[:], cntP[:])

            alpha = sb.tile([1, K], F32)
            nc.vector.tensor_scalar(alpha[:], cntf[:], EPS - 1.0, None, op0=Alu.add)
            nc.vector.reciprocal(alpha[:], alpha[:])
            beta = sb.tile([1, K], F32)
            nc.vector.tensor_scalar(beta[:], cntf[:], -1.0, float(N) + EPS,
                                    op0=Alu.mult, op1=Alu.add)
            nc.vector.reciprocal(beta[:], beta[:])
            vmask = sb.tile([1, K], F32)
            nc.vector.tensor_scalar(vmask[:], cntf[:], 1.5, None, op0=Alu.is_ge)
            nmc = sb.tile([1, K], F32)
            nc.vector.tensor_scalar(nmc[:], cntf[:], -1.0, float(N),
                                    op0=Alu.mult, op1=Alu.add)        # N-cnt
            nc.vector.tensor_tensor(nmc[:], nmc[:], beta[:], op=Alu.mult)
            pf = sb.tile([1, K], F32)
            nc.vector.tensor_tensor(pf[:], cntf[:], alpha[:], op=Alu.mult)
            nc.vector.tensor_tensor(pf[:], pf[:], nmc[:], op=Alu.subtract)

            cpack = sb.tile([1, 3 * K], F32)
            nc.vector.tensor_scalar(cpack[:, 0:K], beta[:], 2.0, 0.0,
                                    op0=Alu.mult, op1=Alu.add)
            nc.vector.tensor_scalar(cpack[:, K:2 * K], alpha[:], -2.0, 0.0,
                                    op0=Alu.mult, op1=Alu.add)
            nc.vector.tensor_tensor(cpack[:, 2 * K:3 * K], pf[:], vmask[:],
                                    op=Alu.mult)                       # P*vm
            vm2 = cpack[:, 0:2 * K].rearrange("o (a c) -> o a c", c=K)
            vmb = vmask.unsqueeze(1).broadcast_to((1, 2, K))
            nc.vector.tensor_tensor(vm2[:, :, :], vm2, vmb, op=Alu.mult)

            # ------------- squares ----------------
            # rows-layout squares on Act (fp8 in -> bf16 out)
            fsqs = []
            TPC = T // FCH
            FC = T * D // FCH
            for g in range(FCH):
                fsq = sb.tile([128, TPC * D], BF16, tag="fsq", bufs=4, name=f"fsq{g}")
                if g < 2:
                    nc.scalar.activation(fsq[:], fh[:, g * FC:(g + 1) * FC],
                                         Act.Square)
                else:
                    nc.vector.tensor_tensor(fsq[:], fh[:, g * FC:(g + 1) * FC],
                                            fh[:, g * FC:(g + 1) * FC],
                                            op=Alu.mult)
                fsqs.append(fsq.rearrange("p (t d) -> p t d", d=D))
            # transposed squares on Vector (fp8 in -> fp8 out)
            ft2 = sb.tile([128, N], FP8)
            FT = N // FCH
            with nc.allow_low_precision(reason="fp8 squares feed P*sq only"):
                for g in (0, 1):
                    nc.vector.tensor_tensor(ft2[:, g * FT:(g + 1) * FT],
                                            ft[:, g * FT:(g + 1) * FT],
                                            ft[:, g * FT:(g + 1) * FT],
                                            op=Alu.mult)
                for g in (2, 3):
                    nc.scalar.activation(ft2[:, g * FT:(g + 1) * FT],
                                         ft[:, g * FT:(g + 1) * FT], Act.Square)

            # ------------- stats + sqstats chains (+ bcast matmul) -------------
            statsP = ps.tile([128, K], F32)
            sqstP = ps.tile([128, K], F32)
            bcP = ps.tile([128, 3 * K], F32)
            for t in range(T):
                nc.tensor.matmul(statsP[:], fh3[:, t, :], eoh83[:, t, :],
                                 start=(t == 0), stop=(t == T - 1),
                                 skip_group_check=True)
                nc.tensor.matmul(sqstP[:], fsqs[t // TPC][:, t % TPC, :],
                                 eohb3[:, t, :],
                                 start=(t == 0), stop=(t == T - 1),
                                 skip_group_check=True)
                if t == 31:
                    nc.tensor.matmul(bcP[:], ones1[:], cpack[:], start=True,
                                     stop=True, skip_group_check=True)

            # ------------- SqS = column sums of sqstats ----------------
            sqstS = sb.tile([128, K], F32)
            nc.vector.tensor_copy(sqstS[:], sqstP[:])
            csP = ps.tile([1, K], F32, tag="smallP", bufs=2, name="csP")
            nc.tensor.matmul(csP[:], ones128[:], sqstS[:], start=True, stop=True,
                             skip_group_check=True)
            SqS = sb.tile([1, K], F32)
            nc.vector.tensor_copy(SqS[:], csP[:])

            # ------------- QM (needs SqS) ----------------
            ssall = sb.tile([1, 1], F32)
            nc.vector.tensor_reduce(ssall[:], SqS[:], axis=AxX, op=Alu.add)
            t1 = sb.tile([1, K], F32)
            nc.scalar.activation(t1[:], SqS[:], Act.Identity, bias=ssall[:],
                                 scale=-1.0)                           # SSall-SqS
            nc.vector.tensor_tensor(t1[:], t1[:], beta[:], op=Alu.mult)
            qm = sb.tile([1, K], F32)
            nc.vector.tensor_tensor(qm[:], SqS[:], alpha[:], op=Alu.mult)
            nc.vector.scalar_tensor_tensor(qm[:], qm[:], MARGIN, t1[:],
                                           op0=Alu.add, op1=Alu.subtract)
            nc.vector.tensor_tensor(qm[:], qm[:], vmask[:], op=Alu.mult)
            qmtP = ps.tile([K, 1], F32, tag="smallP", bufs=2, name="qmtP")
            nc.tensor.matmul(qmtP[:], qm[:], ones1[:, 0:1], start=True, stop=True,
                             skip_group_check=True)
            qm16 = sb.tile([K, 1], F32)
            nc.vector.tensor_copy(qm16[:], qmtP[:])

            # ------------- RT (needs stats) ----------------
            statsS = sb.tile([128, K], F32)
            nc.vector.tensor_copy(statsS[:], statsP[:])
            ftot = sb.tile([128, 1], F32)
            nc.vector.tensor_reduce(ftot[:], statsS[:], axis=AxX, op=Alu.add)
            rtf = sb.tile([128, K], F32)
            nc.scalar.activation(rtf[:], statsS[:], Act.Identity, bias=ftot[:],
                                 scale=-1.0)                           # Ftot-C^T
            nc.vector.tensor_tensor(rtf[:], rtf[:], bcP[:, 0:K], op=Alu.mult)
            tmp2 = sb.tile([128, K], F32)
            nc.vector.tensor_tensor(tmp2[:], statsS[:], bcP[:, K:2 * K], op=Alu.mult)
            rts = sb.tile([128, K], FP8)
            with nc.allow_low_precision(reason="fp8 dot weights, validated"):
                nc.vector.tensor_tensor(rts[:], rtf[:], tmp2[:], op=Alu.add)
                p128s = sb.tile([128, K], FP8)
                nc.vector.tensor_copy(p128s[:], bcP[:, 2 * K:3 * K])

            # ------------- loss chunks ----------------
            partials = sb.tile([K, NCH], F32)
            for k in range(NCH):
                dP = ps.tile([K, CH], F32, tag="dpsum", bufs=2, name=f"dP{k}")
                nc.tensor.matmul(dP[:], rts[:], ft[:, k * CH:(k + 1) * CH],
                                 start=True, stop=False)
                nc.tensor.matmul(dP[:], p128s[:], ft2[:, k * CH:(k + 1) * CH],
                                 start=False, stop=True)
                mskd = sb.tile([K, CH], BF16, tag="mskd", bufs=3, name=f"m{k}")
                nc.scalar.activation(mskd[:], dP[:], Act.Relu, bias=qm16[:])
                scr = sb.tile([K, CH], BF16, tag="scr", bufs=3, name=f"s{k}")
                nc.vector.scalar_tensor_tensor(scr[:], mskd[:], 0.0,
                                               eohts[:, k * CH:(k + 1) * CH],
                                               op0=Alu.add, op1=Alu.mult,
                                               accum_out=partials[:, k:k + 1])

            # ------------- final reduction ----------------
            numP = ps.tile([1, NCH], F32, tag="smallP", bufs=2, name="numP")
            nc.tensor.matmul(numP[:], ones128[0:K, :], partials[:],
                             start=True, stop=True, skip_group_check=True)
            num = sb.tile([1, 1], F32)
            nc.vector.tensor_reduce(num[:], numP[:], axis=AxX, op=Alu.add)
            dv = sb.tile([1, K], F32)
            nc.vector.tensor_tensor(dv[:], cntf[:], vmask[:], op=Alu.mult)
            den = sb.tile([1, 1], F32)
            nc.vector.tensor_reduce(den[:], dv[:], axis=AxX, op=Alu.add)
            nc.vector.tensor_scalar(den[:], den[:], 1.0, None, op0=Alu.max)
            nc.vector.reciprocal(den[:], den[:])
            resS = sb.tile([1, 1], F32)
            nc.vector.tensor_tensor(resS[:], num[:], den[:], op=Alu.mult)
            nc.sync.dma_start(res, resS[:])

    nc.compile()
    _CACHE["nc"] = nc
    return nc


def _make_in_maps(features, labels):
    feats = np.ascontiguousarray(np.asarray(features, dtype=np.float32))
    lab = np.ascontiguousarray(np.asarray(labels)).astype(np.int64)
    bf = ml_dtypes.bfloat16
    f8 = ml_dtypes.float8_e4m3

    oh = lab[:, None] == np.arange(K, dtype=np.int64)[None, :]          # (N, K)
    ohr = oh.reshape(T, 128, K).transpose(1, 0, 2).reshape(128, T * K)
    fhrows = feats.reshape(T, 128, D).transpose(1, 0, 2).reshape(128, T * D)
    one = {
        "fhr": np.ascontiguousarray(fhrows).astype(f8),
        "ftr": np.ascontiguousarray(feats.T).astype(f8),
        "eohr8": np.ascontiguousarray(ohr).astype(f8),
        "eohrb": np.ascontiguousarray(ohr).astype(bf),
        "eoht": np.ascontiguousarray(oh.T).astype(bf),
    }
    return [dict(one) for _ in range(NCORES)]


def kernel(features, labels):
    nc = _build()
    in_maps = _make_in_maps(features, labels)
    out = run_bass_kernel_spmd(nc, in_maps, core_ids=list(range(NCORES)))
    return np.float32(out.results[0]["res"][0, 0])


# revision 16
# speedup vs baseline: 1.0829x; 1.0829x over previous
"""Trainium2 Bass kernel for nn_ContrastiveDist (supervised contrastive loss).

Math
----
The (n,n) distance/weight matrices collapse to per-class statistics.  With
classes c = 0..15, per-class count cnt[c], feature sums C[c,:], squared-norm
sums SqS[c], global sums Ftot / SSall:

    alpha[c] = 1/(cnt[c]-1+eps),  beta[c] = 1/(n-cnt[c]+eps)
    loss_i   = f_i . R[c_i] + sq_i*P[c_i] + (Q[c_i]+M)
      R[c,:] = 2*beta*(Ftot-C[c]) - 2*alpha*C[c]
      P[c]   = alpha*cnt - beta*(n-cnt)
      Q[c]   = alpha*SqS[c] - beta*(SSall-SqS[c])
    result   = sum(relu(loss_i)*valid_i) / max(sum(valid_i), 1)

valid_i = (cnt[c_i] >= 2) is folded into the coefficients (R/P/QM rows of
invalid classes zeroed -> relu(loss)=0 there).

Device pipeline (fp8e4 features, ~1e-4 rel err vs f32 reference; errors are
row/element-wise symmetric roundings that average out over 8192 rows):
  1. two interleaved 64-matmul PSUM chains over the row tiles produce
     statsT(128d,16c) = sum_t fh_t^T @ onehot_t  (fp8 x fp8) and
     sqstatsT(128d,16c) = sum_t (fh_t^2)^T @ onehot_t  (bf16 x bf16),
     overlapped with the feature DMA.
  2. cnt-only coefficients (alpha/beta/vmask/P and their 128-partition
     broadcast via a ones(1,128) rank-1 matmul) are computed EARLY from the
     one-hot column sums; only QM (SqS) and RT (stats) trail the DMA.
  3. loss:   per 512-col chunk, PSUM = RT^T @ fT + P128^T @ fT^2 (all fp8;
     the second matmul realizes P[c]*sq_i since sum_d fT^2[d,i] = sq_i),
     then relu(PSUM + QM[c]) on the scalar engine and mask*accumulate on
     vector.
HBM traffic ~2.8MB/core; every core computes redundantly (no collectives).
Notes: DVE (MULTIPLY, BYPASS) tensor_scalar is pathologically slow; DVE
stride-0 broadcast operands are ~100x slow (use Act per-partition bias
instead); dma_start stalls the issuing sequencer while its HWDGE ring is
busy, so the Act ring gets exactly one transfer and Act's compute starts
only after that single trigger.
"""

import numpy as np
import ml_dtypes

import concourse.bacc as bacc
import concourse.tile as tile
import concourse.mybir as mybir
from concourse.bass_utils import run_bass_kernel_spmd

N, D, K, NCORES = 8192, 128, 16, 8
T = N // 128               # 64 row-tiles of 128
NCH = 16                   # dot chunks of 512 cols
CH = N // NCH
FCH = 4                    # square chunking (2048 cols each)
EPS, MARGIN = 1e-6, 10.0
F32 = mybir.dt.float32
BF16 = mybir.dt.bfloat16
FP8 = mybir.dt.float8e4
Alu = mybir.AluOpType
Act = mybir.ActivationFunctionType
AxX = mybir.AxisListType.X

_CACHE: dict = {}


def _build():
    if "nc" in _CACHE:
        return _CACHE["nc"]

    nc = bacc.Bacc("TRN2", target_bir_lowering=False, debug=False, num_devices=NCORES)
    fhr = nc.dram_tensor("fhr", [128, T * D], FP8, kind="ExternalInput").ap()
    ftr = nc.dram_tensor("ftr", [128, N], FP8, kind="ExternalInput").ap()
    eohr8 = nc.dram_tensor("eohr8", [128, T * K], FP8, kind="ExternalInput").ap()
    eohrb = nc.dram_tensor("eohrb", [128, T * K], BF16, kind="ExternalInput").ap()
    eoht = nc.dram_tensor("eoht", [128, N // 2], FP8, kind="ExternalInput").ap()
    res = nc.dram_tensor("res", [1, 1], F32, kind="ExternalOutput").ap()

    with tile.TileContext(nc) as tc:
        with (
            tc.tile_pool(name="sb", bufs=1) as sb,
            tc.tile_pool(name="ps", bufs=1, space="PSUM") as ps,
        ):
            # ------------- loads: sync + gpsimd rings, one DMA on Act ring ------
            eohb = sb.tile([128, T * K], BF16)
            eoh8 = sb.tile([128, T * K], FP8)
            fh = sb.tile([128, T * D], FP8)
            ft = sb.tile([128, N], FP8)
            eohts = sb.tile([128, N // 2], FP8)
            HF = T * D // 2
            HT = N // 2
            # Act ring: labels + first ft chunk; Act compute starts at ~fh
            # arrival anyway, so the trigger stalls cost nothing.
            QT = N // 4
            nc.scalar.dma_start(eohb[:], eohrb)
            nc.scalar.dma_start(eoh8[:], eohr8)
            nc.scalar.dma_start(ft[:, 0:QT], ftr[:, 0:QT])
            nc.scalar.dma_start(eohts[:], eoht)
            nc.sync.dma_start(fh[:, 0:HF], fhr[:, 0:HF])
            nc.gpsimd.dma_start(fh[:, HF:2 * HF], fhr[:, HF:2 * HF])
            nc.sync.dma_start(ft[:, QT:3 * QT], ftr[:, QT:3 * QT])
            nc.gpsimd.dma_start(ft[:, 3 * QT:4 * QT], ftr[:, 3 * QT:4 * QT])

            fh3 = fh.rearrange("p (t d) -> p t d", d=D)
            eoh83 = eoh8.rearrange("p (t c) -> p t c", c=K)
            eohb3 = eohb.rearrange("p (t c) -> p t c", c=K)
            eohb3c = eohb.rearrange("p (t c) -> p c t", c=K)

            ones128 = sb.tile([128, 1], F32)
            nc.gpsimd.memset(ones128[:], 1.0)
            ones1 = sb.tile([1, 128], F32)
            nc.gpsimd.memset(ones1[:], 1.0)
            # preload the Relu activation table off the critical path
            dumm = sb.tile([1, 1], BF16)
            nc.scalar.activation(dumm[:], ones1[:, 0:1], Act.Relu)

            # ------------- early: cnt and cnt-only coefficients ----------------
            cntpart = sb.tile([128, K], F32)
            nc.vector.tensor_reduce(cntpart[:], eohb3c, axis=AxX, op=Alu.add)
            cntP = ps.tile([1, K], F32, tag="cntP", bufs=1, name="cntP")
            nc.tensor.matmul(cntP[:], ones128[:], cntpart[:], start=True, stop=True,
                             skip_group_check=True)
            cntf = sb.tile([1, K], F32)
            nc.vector.tensor_copy(cntf[:], cntP[:])

            alpha = sb.tile([1, K], F32)
            nc.vector.tensor_scalar(alpha[:], cntf[:], EPS - 1.0, None, op0=Alu.add)
            nc.vector.reciprocal(alpha[:], alpha[:])
            beta = sb.tile([1, K], F32)
            nc.vector.tensor_scalar(beta[:], cntf[:], -1.0, float(N) + EPS,
                                    op0=Alu.mult, op1=Alu.add)
            nc.vector.reciprocal(beta[:], beta[:])
            vmask = sb.tile([1, K], F32)
            nc.vector.tensor_scalar(vmask[:], cntf[:], 1.5, None, op0=Alu.is_ge)
            nmc = sb.tile([1, K], F32)
            nc.vector.tensor_scalar(nmc[:], cntf[:], -1.0, float(N),
                                    op0=Alu.mult, op1=Alu.add)        # N-cnt
            nc.vector.tensor_tensor(nmc[:], nmc[:], beta[:], op=Alu.mult)
            pf = sb.tile([1, K], F32)
            nc.vector.tensor_tensor(pf[:], cntf[:], alpha[:], op=Alu.mult)
            nc.vector.tensor_tensor(pf[:], pf[:], nmc[:], op=Alu.subtract)

            cpack = sb.tile([1, 3 * K], F32)
            nc.vector.tensor_scalar(cpack[:, 0:K], beta[:], 2.0, 0.0,
                                    op0=Alu.mult, op1=Alu.add)
            nc.vector.tensor_scalar(cpack[:, K:2 * K], alpha[:], -2.0, 0.0,
                                    op0=Alu.mult, op1=Alu.add)
            nc.vector.tensor_tensor(cpack[:, 2 * K:3 * K], pf[:], vmask[:],
                                    op=Alu.mult)                       # P*vm
            vm2 = cpack[:, 0:2 * K].rearrange("o (a c) -> o a c", c=K)
            vmb = vmask.unsqueeze(1).broadcast_to((1, 2, K))
            nc.vector.tensor_tensor(vm2[:, :, :], vm2, vmb, op=Alu.mult)

            # ------------- squares ----------------
            # rows-layout squares on Act (fp8 in -> bf16 out)
            fsqs = []
            TPC = T // FCH
            FC = T * D // FCH
            for g in range(FCH):
                fsq = sb.tile([128, TPC * D], BF16, tag="fsq", bufs=4, name=f"fsq{g}")
                if g < 2:
                    nc.scalar.activation(fsq[:], fh[:, g * FC:(g + 1) * FC],
                                         Act.Square)
                else:
                    nc.vector.tensor_tensor(fsq[:], fh[:, g * FC:(g + 1) * FC],
                                            fh[:, g * FC:(g + 1) * FC],
                                            op=Alu.mult)
                fsqs.append(fsq.rearrange("p (t d) -> p t d", d=D))
            # transposed squares on Vector (fp8 in -> fp8 out)
            ft2 = sb.tile([128, N], FP8)
            FT = N // FCH
            with nc.allow_low_precision(reason="fp8 squares feed P*sq only"):
                for g in (0, 1):
                    nc.vector.tensor_tensor(ft2[:, g * FT:(g + 1) * FT],
                                            ft[:, g * FT:(g + 1) * FT],
                                            ft[:, g * FT:(g + 1) * FT],
                                            op=Alu.mult)
                for g in (2, 3):
                    nc.scalar.activation(ft2[:, g * FT:(g + 1) * FT],
                                         ft[:, g * FT:(g + 1) * FT], Act.Square)

            # ------------- stats + sqstats chains (+ bcast matmul) -------------
            statsP = ps.tile([128, K], F32)
            sqstP = ps.tile([128, K], F32)
            bcP = ps.tile([128, 3 * K], F32)
            for t in range(T):
                nc.tensor.matmul(statsP[:], fh3[:, t, :], eoh83[:, t, :],
                                 start=(t == 0), stop=(t == T - 1),
                                 skip_group_check=True)
                nc.tensor.matmul(sqstP[:], fsqs[t // TPC][:, t % TPC, :],
                                 eohb3[:, t, :],
                                 start=(t == 0), stop=(t == T - 1),
                                 skip_group_check=True)
                if t == 31:
                    nc.tensor.matmul(bcP[:], ones1[:], cpack[:], start=True,
                                     stop=True, skip_group_check=True)

            # ------------- SqS = column sums of sqstats ----------------
            sqstS = sb.tile([128, K], F32)
            nc.vector.tensor_copy(sqstS[:], sqstP[:])
            csP = ps.tile([1, K], F32, tag="smallP", bufs=2, name="csP")
            nc.tensor.matmul(csP[:], ones128[:], sqstS[:], start=True, stop=True,
                             skip_group_check=True)
            SqS = sb.tile([1, K], F32)
            nc.vector.tensor_copy(SqS[:], csP[:])

            # ------------- QM (needs SqS) ----------------
            ssall = sb.tile([1, 1], F32)
            nc.vector.tensor_reduce(ssall[:], SqS[:], axis=AxX, op=Alu.add)
            t1 = sb.tile([1, K], F32)
            nc.scalar.activation(t1[:], SqS[:], Act.Identity, bias=ssall[:],
                                 scale=-1.0)                           # SSall-SqS
            nc.vector.tensor_tensor(t1[:], t1[:], beta[:], op=Alu.mult)
            qm = sb.tile([1, K], F32)
            nc.vector.tensor_tensor(qm[:], SqS[:], alpha[:], op=Alu.mult)
            nc.vector.scalar_tensor_tensor(qm[:], qm[:], MARGIN, t1[:],
                                           op0=Alu.add, op1=Alu.subtract)
            nc.vector.tensor_tensor(qm[:], qm[:], vmask[:], op=Alu.mult)

            # ------------- RT (needs stats) ----------------
            statsS = sb.tile([128, K], F32)
            nc.vector.tensor_copy(statsS[:], statsP[:])
            ftot = sb.tile([128, 1], F32)
            nc.vector.tensor_reduce(ftot[:], statsS[:], axis=AxX, op=Alu.add)
            rtf = sb.tile([128, K], F32)
            nc.scalar.activation(rtf[:], statsS[:], Act.Identity, bias=ftot[:],
                                 scale=-1.0)                           # Ftot-C^T
            nc.vector.tensor_tensor(rtf[:], rtf[:], bcP[:, 0:K], op=Alu.mult)
            tmp2 = sb.tile([128, K], F32)
            nc.vector.tensor_tensor(tmp2[:], statsS[:], bcP[:, K:2 * K], op=Alu.mult)
            rts = sb.tile([128, 4 * K], FP8)
            nc.gpsimd.memset(rts[:], 0.0)
            p128s = sb.tile([128, 4 * K], FP8)
            nc.gpsimd.memset(p128s[:], 0.0)
            with nc.allow_low_precision(reason="fp8 dot weights, validated"):
                nc.vector.tensor_tensor(rts[:, 0:K], rtf[:], tmp2[:], op=Alu.add)
                nc.vector.tensor_copy(p128s[:, 0:K], bcP[:, 2 * K:3 * K])
            # relu bias replicated to the two 64-partition groups (gaps = 0)
            qm128 = sb.tile([128, 1], F32)
            nc.gpsimd.memset(qm128[:], 0.0)
            for g in range(2):
                qgP = ps.tile([K, 1], F32, tag="smallP", bufs=2, name=f"qg{g}")
                nc.tensor.matmul(qgP[:], qm[:], ones1[:, 0:1], start=True,
                                 stop=True, skip_group_check=True)
                nc.vector.tensor_copy(qm128[64 * g:64 * g + K, :], qgP[:])

            # ------------- loss rounds: 2 chunks packed per PSUM tile ----------
            partials = sb.tile([128, 8], F32)
            for r in range(8):
                dP = ps.tile([128, CH], F32, tag="dpsum", bufs=2, name=f"dP{r}")
                for g in range(2):
                    k = 2 * r + g
                    nc.tensor.matmul(dP[64 * g:64 * g + 64, :], rts[:],
                                     ft[:, k * CH:(k + 1) * CH],
                                     start=True, stop=False,
                                     skip_group_check=True)
                    nc.tensor.matmul(dP[64 * g:64 * g + 64, :], p128s[:],
                                     ft2[:, k * CH:(k + 1) * CH],
                                     start=False, stop=True,
                                     skip_group_check=True)
                mskd = sb.tile([128, CH], BF16, tag="mskd", bufs=3, name=f"m{r}")
                nc.scalar.activation(mskd[:], dP[:], Act.Relu, bias=qm128[:])
                scr = sb.tile([128, CH], BF16, tag="scr", bufs=3, name=f"s{r}")
                nc.vector.scalar_tensor_tensor(scr[:], mskd[:], 0.0,
                                               eohts[:, r * CH:(r + 1) * CH],
                                               op0=Alu.add, op1=Alu.mult,
                                               accum_out=partials[:, r:r + 1])

            # ------------- final reduction ----------------
            numP = ps.tile([1, 8], F32, tag="smallP", bufs=2, name="numP")
            nc.tensor.matmul(numP[:], ones128[:], partials[:],
                             start=True, stop=True, skip_group_check=True)
            num = sb.tile([1, 1], F32)
            nc.vector.tensor_reduce(num[:], numP[:], axis=AxX, op=Alu.add)
            dv = sb.tile([1, K], F32)
            nc.vector.tensor_tensor(dv[:], cntf[:], vmask[:], op=Alu.mult)
            den = sb.tile([1, 1], F32)
            nc.vector.tensor_reduce(den[:], dv[:], axis=AxX, op=Alu.add)
            nc.vector.tensor_scalar(den[:], den[:], 1.0, None, op0=Alu.max)
            nc.vector.reciprocal(den[:], den[:])
            resS = sb.tile([1, 1], F32)
            nc.vector.tensor_tensor(resS[:], num[:], den[:], op=Alu.mult)
            nc.sync.dma_start(res, resS[:])

    nc.compile()
    _CACHE["nc"] = nc
    return nc


def _make_in_maps(features, labels):
    feats = np.ascontiguousarray(np.asarray(features, dtype=np.float32))
    lab = np.ascontiguousarray(np.asarray(labels)).astype(np.int64)
    bf = ml_dtypes.bfloat16
    f8 = ml_dtypes.float8_e4m3

    oh = lab[:, None] == np.arange(K, dtype=np.int64)[None, :]          # (N, K)
    ohT = oh.T.reshape(K, 16, 512)                  # (16c, 16chunks, 512)
    ohp = np.zeros((128, 4096), dtype=np.float32)   # 8 rounds of 2 packed chunks
    for r in range(8):
        for g in range(2):
            ohp[64 * g:64 * g + K, r * 512:(r + 1) * 512] = ohT[:, 2 * r + g, :]
    ohr = oh.reshape(T, 128, K).transpose(1, 0, 2).reshape(128, T * K)
    fhrows = feats.reshape(T, 128, D).transpose(1, 0, 2).reshape(128, T * D)
    one = {
        "fhr": np.ascontiguousarray(fhrows).astype(f8),
        "ftr": np.ascontiguousarray(feats.T).astype(f8),
        "eohr8": np.ascontiguousarray(ohr).astype(f8),
        "eohrb": np.ascontiguousarray(ohr).astype(bf),
        "eoht": np.ascontiguousarray(ohp).astype(f8),
    }
    return [dict(one) for _ in range(NCORES)]


def kernel(features, labels):
    nc = _build()
    in_maps = _make_in_maps(features, labels)
    out = run_bass_kernel_spmd(nc, in_maps, core_ids=list(range(NCORES)))
    return np.float32(out.results[0]["res"][0, 0])


# revision 17
# speedup vs baseline: 1.0980x; 1.0139x over previous
"""Trainium2 Bass kernel for nn_ContrastiveDist (supervised contrastive loss).

Math
----
The (n,n) distance/weight matrices collapse to per-class statistics.  With
classes c = 0..15, per-class count cnt[c], feature sums C[c,:], squared-norm
sums SqS[c], global sums Ftot / SSall:

    alpha[c] = 1/(cnt[c]-1+eps),  beta[c] = 1/(n-cnt[c]+eps)
    loss_i   = f_i . R[c_i] + sq_i*P[c_i] + (Q[c_i]+M)
      R[c,:] = 2*beta*(Ftot-C[c]) - 2*alpha*C[c]
      P[c]   = alpha*cnt - beta*(n-cnt)
      Q[c]   = alpha*SqS[c] - beta*(SSall-SqS[c])
    result   = sum(relu(loss_i)*valid_i) / max(sum(valid_i), 1)

valid_i = (cnt[c_i] >= 2) is folded into the coefficients (R/P/QM rows of
invalid classes zeroed -> relu(loss)=0 there).

Device pipeline (fp8e4 features, ~1e-4 rel err vs f32 reference; errors are
row/element-wise symmetric roundings that average out over 8192 rows):
  1. two interleaved 64-matmul PSUM chains over the row tiles produce
     statsT(128d,16c) = sum_t fh_t^T @ onehot_t  (fp8 x fp8) and
     sqstatsT(128d,16c) = sum_t (fh_t^2)^T @ onehot_t  (bf16 x bf16),
     overlapped with the feature DMA.
  2. cnt-only coefficients (alpha/beta/vmask/P and their 128-partition
     broadcast via a ones(1,128) rank-1 matmul) are computed EARLY from the
     one-hot column sums; only QM (SqS) and RT (stats) trail the DMA.
  3. loss:   per 512-col chunk, PSUM = RT^T @ fT + P128^T @ fT^2 (all fp8;
     the second matmul realizes P[c]*sq_i since sum_d fT^2[d,i] = sq_i),
     then relu(PSUM + QM[c]) on the scalar engine and mask*accumulate on
     vector.
HBM traffic ~2.8MB/core; every core computes redundantly (no collectives).
Notes: DVE (MULTIPLY, BYPASS) tensor_scalar is pathologically slow; DVE
stride-0 broadcast operands are ~100x slow (use Act per-partition bias
instead); dma_start stalls the issuing sequencer while its HWDGE ring is
busy, so the Act ring gets exactly one transfer and Act's compute starts
only after that single trigger.
"""

import numpy as np
import ml_dtypes

import concourse.bacc as bacc
import concourse.tile as tile
import concourse.mybir as mybir
from concourse.bass_utils import run_bass_kernel_spmd

N, D, K, NCORES = 8192, 128, 16, 8
T = N // 128               # 64 row-tiles of 128
NCH = 16                   # dot chunks of 512 cols
CH = N // NCH
FCH = 4                    # square chunking (2048 cols each)
EPS, MARGIN = 1e-6, 10.0
F32 = mybir.dt.float32
BF16 = mybir.dt.bfloat16
FP8 = mybir.dt.float8e4
Alu = mybir.AluOpType
Act = mybir.ActivationFunctionType
AxX = mybir.AxisListType.X

_CACHE: dict = {}


def _build():
    if "nc" in _CACHE:
        return _CACHE["nc"]

    nc = bacc.Bacc("TRN2", target_bir_lowering=False, debug=False, num_devices=NCORES)
    fhr = nc.dram_tensor("fhr", [128, T * D], FP8, kind="ExternalInput").ap()
    ftr = nc.dram_tensor("ftr", [128, N], FP8, kind="ExternalInput").ap()
    eohr8 = nc.dram_tensor("eohr8", [128, T * K], FP8, kind="ExternalInput").ap()
    eohrb = nc.dram_tensor("eohrb", [128, T * K], BF16, kind="ExternalInput").ap()
    eoht = nc.dram_tensor("eoht", [128, N // 2], FP8, kind="ExternalInput").ap()
    res = nc.dram_tensor("res", [1, 1], F32, kind="ExternalOutput").ap()

    with tile.TileContext(nc) as tc:
        with (
            tc.tile_pool(name="sb", bufs=1) as sb,
            tc.tile_pool(name="ps", bufs=1, space="PSUM") as ps,
        ):
            # ------------- loads: sync + gpsimd rings, one DMA on Act ring ------
            eohb = sb.tile([128, T * K], BF16)
            eoh8 = sb.tile([128, T * K], FP8)
            fh = sb.tile([128, T * D], FP8)
            ft = sb.tile([128, N], FP8)
            eohts = sb.tile([128, N // 2], FP8)
            HF = T * D // 2
            HT = N // 2
            # Act ring: labels + first ft chunk; Act compute starts at ~fh
            # arrival anyway, so the trigger stalls cost nothing.
            QT = N // 4
            nc.scalar.dma_start(eohb[:], eohrb)
            nc.scalar.dma_start(eoh8[:], eohr8)
            nc.scalar.dma_start(ft[:, 0:QT], ftr[:, 0:QT])
            nc.scalar.dma_start(eohts[:], eoht)
            nc.sync.dma_start(fh[:, 0:HF], fhr[:, 0:HF])
            nc.gpsimd.dma_start(fh[:, HF:2 * HF], fhr[:, HF:2 * HF])
            nc.sync.dma_start(ft[:, QT:3 * QT], ftr[:, QT:3 * QT])
            nc.gpsimd.dma_start(ft[:, 3 * QT:4 * QT], ftr[:, 3 * QT:4 * QT])

            fh3 = fh.rearrange("p (t d) -> p t d", d=D)
            eoh83 = eoh8.rearrange("p (t c) -> p t c", c=K)
            eohb3 = eohb.rearrange("p (t c) -> p t c", c=K)
            eohb3c = eohb.rearrange("p (t c) -> p c t", c=K)

            ones128 = sb.tile([128, 1], F32)
            nc.gpsimd.memset(ones128[:], 1.0)
            ones1 = sb.tile([1, 128], F32)
            nc.gpsimd.memset(ones1[:], 1.0)
            # preload the Relu activation table off the critical path
            dumm = sb.tile([1, 1], BF16)
            nc.scalar.activation(dumm[:], ones1[:, 0:1], Act.Relu)

            # ------------- early: cnt and cnt-only coefficients ----------------
            cntpart = sb.tile([128, K], F32)
            nc.vector.tensor_reduce(cntpart[:], eohb3c, axis=AxX, op=Alu.add)
            cntP = ps.tile([1, K], F32, tag="cntP", bufs=1, name="cntP")
            nc.tensor.matmul(cntP[:], ones128[:], cntpart[:], start=True, stop=True,
                             skip_group_check=True)
            cntf = sb.tile([1, K], F32)
            nc.vector.tensor_copy(cntf[:], cntP[:])

            alpha = sb.tile([1, K], F32)
            nc.vector.tensor_scalar(alpha[:], cntf[:], EPS - 1.0, None, op0=Alu.add)
            nc.vector.reciprocal(alpha[:], alpha[:])
            beta = sb.tile([1, K], F32)
            nc.vector.tensor_scalar(beta[:], cntf[:], -1.0, float(N) + EPS,
                                    op0=Alu.mult, op1=Alu.add)
            nc.vector.reciprocal(beta[:], beta[:])
            vmask = sb.tile([1, K], F32)
            nc.vector.tensor_scalar(vmask[:], cntf[:], 1.5, None, op0=Alu.is_ge)
            nmc = sb.tile([1, K], F32)
            nc.vector.tensor_scalar(nmc[:], cntf[:], -1.0, float(N),
                                    op0=Alu.mult, op1=Alu.add)        # N-cnt
            nc.vector.tensor_tensor(nmc[:], nmc[:], beta[:], op=Alu.mult)
            pf = sb.tile([1, K], F32)
            nc.vector.tensor_tensor(pf[:], cntf[:], alpha[:], op=Alu.mult)
            nc.vector.tensor_tensor(pf[:], pf[:], nmc[:], op=Alu.subtract)

            cpack = sb.tile([1, 3 * K], F32)
            nc.vector.tensor_scalar(cpack[:, 0:K], beta[:], 2.0, 0.0,
                                    op0=Alu.mult, op1=Alu.add)
            nc.vector.tensor_scalar(cpack[:, K:2 * K], alpha[:], -2.0, 0.0,
                                    op0=Alu.mult, op1=Alu.add)
            nc.vector.tensor_tensor(cpack[:, 2 * K:3 * K], pf[:], vmask[:],
                                    op=Alu.mult)                       # P*vm
            vm2 = cpack[:, 0:2 * K].rearrange("o (a c) -> o a c", c=K)
            vmb = vmask.unsqueeze(1).broadcast_to((1, 2, K))
            nc.vector.tensor_tensor(vm2[:, :, :], vm2, vmb, op=Alu.mult)

            # ------------- squares ----------------
            # rows-layout squares on Act (fp8 in -> bf16 out)
            fsqs = []
            TPC = T // FCH
            FC = T * D // FCH
            for g in range(FCH):
                fsq = sb.tile([128, TPC * D], BF16, tag="fsq", bufs=4, name=f"fsq{g}")
                nc.scalar.activation(fsq[:], fh[:, g * FC:(g + 1) * FC],
                                     Act.Square)
                fsqs.append(fsq.rearrange("p (t d) -> p t d", d=D))
            # transposed squares on Vector (fp8 in -> fp8 out)
            ft2 = sb.tile([128, N], FP8)
            FT = N // FCH
            with nc.allow_low_precision(reason="fp8 squares feed P*sq only"):
                for g in range(FCH):
                    nc.vector.tensor_tensor(ft2[:, g * FT:(g + 1) * FT],
                                            ft[:, g * FT:(g + 1) * FT],
                                            ft[:, g * FT:(g + 1) * FT],
                                            op=Alu.mult)

            # ------------- stats + sqstats chains (+ bcast matmul) -------------
            statsP = ps.tile([128, K], F32)
            sqstP = ps.tile([128, K], F32)
            bcP = ps.tile([128, 3 * K], F32)
            for t in range(T):
                nc.tensor.matmul(statsP[:], fh3[:, t, :], eoh83[:, t, :],
                                 start=(t == 0), stop=(t == T - 1),
                                 skip_group_check=True)
                nc.tensor.matmul(sqstP[:], fsqs[t // TPC][:, t % TPC, :],
                                 eohb3[:, t, :],
                                 start=(t == 0), stop=(t == T - 1),
                                 skip_group_check=True)
                if t == 31:
                    nc.tensor.matmul(bcP[:], ones1[:], cpack[:], start=True,
                                     stop=True, skip_group_check=True)

            # ------------- SqS = column sums of sqstats ----------------
            sqstS = sb.tile([128, K], F32)
            nc.vector.tensor_copy(sqstS[:], sqstP[:])
            csP = ps.tile([1, K], F32, tag="smallP", bufs=1, name="csP")
            nc.tensor.matmul(csP[:], ones128[:], sqstS[:], start=True, stop=True,
                             skip_group_check=True)
            SqS = sb.tile([1, K], F32)
            nc.vector.tensor_copy(SqS[:], csP[:])

            # ------------- QM (needs SqS) ----------------
            ssall = sb.tile([1, 1], F32)
            nc.vector.tensor_reduce(ssall[:], SqS[:], axis=AxX, op=Alu.add)
            t1 = sb.tile([1, K], F32)
            nc.scalar.activation(t1[:], SqS[:], Act.Identity, bias=ssall[:],
                                 scale=-1.0)                           # SSall-SqS
            nc.vector.tensor_tensor(t1[:], t1[:], beta[:], op=Alu.mult)
            qm = sb.tile([1, K], F32)
            nc.vector.tensor_tensor(qm[:], SqS[:], alpha[:], op=Alu.mult)
            nc.vector.scalar_tensor_tensor(qm[:], qm[:], MARGIN, t1[:],
                                           op0=Alu.add, op1=Alu.subtract)
            nc.vector.tensor_tensor(qm[:], qm[:], vmask[:], op=Alu.mult)

            # ------------- RT (needs stats) ----------------
            statsS = sb.tile([128, K], F32)
            nc.vector.tensor_copy(statsS[:], statsP[:])
            ftot = sb.tile([128, 1], F32)
            nc.vector.tensor_reduce(ftot[:], statsS[:], axis=AxX, op=Alu.add)
            rtf = sb.tile([128, K], F32)
            nc.scalar.activation(rtf[:], statsS[:], Act.Identity, bias=ftot[:],
                                 scale=-1.0)                           # Ftot-C^T
            nc.vector.tensor_tensor(rtf[:], rtf[:], bcP[:, 0:K], op=Alu.mult)
            tmp2 = sb.tile([128, K], F32)
            nc.vector.tensor_tensor(tmp2[:], statsS[:], bcP[:, K:2 * K], op=Alu.mult)
            rts = sb.tile([128, 4 * K], FP8)
            nc.gpsimd.memset(rts[:], 0.0)
            p128s = sb.tile([128, 4 * K], FP8)
            nc.gpsimd.memset(p128s[:], 0.0)
            with nc.allow_low_precision(reason="fp8 dot weights, validated"):
                nc.vector.tensor_tensor(rts[:, 0:K], rtf[:], tmp2[:], op=Alu.add)
                nc.vector.tensor_copy(p128s[:, 0:K], bcP[:, 2 * K:3 * K])
            # relu bias replicated to the two 64-partition groups (gaps = 0)
            qm128 = sb.tile([128, 1], F32)
            nc.gpsimd.memset(qm128[:], 0.0)
            for g in range(2):
                qgP = ps.tile([K, 1], F32, tag="smallP", bufs=1, name=f"qg{g}")
                nc.tensor.matmul(qgP[:], qm[:], ones1[:, 0:1], start=True,
                                 stop=True, skip_group_check=True)
                nc.vector.tensor_copy(qm128[64 * g:64 * g + K, :], qgP[:])

            # ------------- loss rounds: 2 chunks packed per PSUM tile ----------
            partials = sb.tile([128, 8], F32)
            for r in range(8):
                dP = ps.tile([128, CH], F32, tag="dpsum", bufs=3, name=f"dP{r}")
                for g in range(2):
                    k = 2 * r + g
                    nc.tensor.matmul(dP[64 * g:64 * g + 64, :], rts[:],
                                     ft[:, k * CH:(k + 1) * CH],
                                     start=True, stop=False,
                                     skip_group_check=True)
                    nc.tensor.matmul(dP[64 * g:64 * g + 64, :], p128s[:],
                                     ft2[:, k * CH:(k + 1) * CH],
                                     start=False, stop=True,
                                     skip_group_check=True)
                mskd = sb.tile([128, CH], BF16, tag="mskd", bufs=3, name=f"m{r}")
                nc.scalar.activation(mskd[:], dP[:], Act.Relu, bias=qm128[:])
                scr = sb.tile([128, CH], BF16, tag="scr", bufs=3, name=f"s{r}")
                nc.vector.scalar_tensor_tensor(scr[:], mskd[:], 0.0,
                                               eohts[:, r * CH:(r + 1) * CH],
                                               op0=Alu.add, op1=Alu.mult,
                                               accum_out=partials[:, r:r + 1])

            # ------------- final reduction ----------------
            numP = ps.tile([1, 8], F32, tag="smallP", bufs=1, name="numP")
            nc.tensor.matmul(numP[:], ones128[:], partials[:],
                             start=True, stop=True, skip_group_check=True)
            num = sb.tile([1, 1], F32)
            nc.vector.tensor_reduce(num[:], numP[:], axis=AxX, op=Alu.add)
            dv = sb.tile([1, K], F32)
            nc.vector.tensor_tensor(dv[:], cntf[:], vmask[:], op=Alu.mult)
            den = sb.tile([1, 1], F32)
            nc.vector.tensor_reduce(den[:], dv[:], axis=AxX, op=Alu.add)
            nc.vector.tensor_scalar(den[:], den[:], 1.0, None, op0=Alu.max)
            nc.vector.reciprocal(den[:], den[:])
            resS = sb.tile([1, 1], F32)
            nc.vector.tensor_tensor(resS[:], num[:], den[:], op=Alu.mult)
            nc.sync.dma_start(res, resS[:])

    nc.compile()
    _CACHE["nc"] = nc
    return nc


def _make_in_maps(features, labels):
    feats = np.ascontiguousarray(np.asarray(features, dtype=np.float32))
    lab = np.ascontiguousarray(np.asarray(labels)).astype(np.int64)
    bf = ml_dtypes.bfloat16
    f8 = ml_dtypes.float8_e4m3

    oh = lab[:, None] == np.arange(K, dtype=np.int64)[None, :]          # (N, K)
    ohT = oh.T.reshape(K, 16, 512)                  # (16c, 16chunks, 512)
    ohp = np.zeros((128, 4096), dtype=np.float32)   # 8 rounds of 2 packed chunks
    for r in range(8):
        for g in range(2):
            ohp[64 * g:64 * g + K, r * 512:(r + 1) * 512] = ohT[:, 2 * r + g, :]
    ohr = oh.reshape(T, 128, K).transpose(1, 0, 2).reshape(128, T * K)
    fhrows = feats.reshape(T, 128, D).transpose(1, 0, 2).reshape(128, T * D)
    one = {
        "fhr": np.ascontiguousarray(fhrows).astype(f8),
        "ftr": np.ascontiguousarray(feats.T).astype(f8),
        "eohr8": np.ascontiguousarray(ohr).astype(f8),
        "eohrb": np.ascontiguousarray(ohr).astype(bf),
        "eoht": np.ascontiguousarray(ohp).astype(f8),
    }
    return [dict(one) for _ in range(NCORES)]


def kernel(features, labels):
    nc = _build()
    in_maps = _make_in_maps(features, labels)
    out = run_bass_kernel_spmd(nc, in_maps, core_ids=list(range(NCORES)))
    return np.float32(out.results[0]["res"][0, 0])
